# revision 12
# baseline (speedup 1.0000x reference)
"""v2 sharded k-NN kernel: fp8 DoubleRow block-diagonal matmul + split pooling.

Per core (125k of 1M memory rows):
  - moving tensor fp8e4m3 [68, 126976]: each psum column packs TWO memory
    rows (parity g in {0,1} -> psum partition groups 0:64 / 64:128) via a
    block-diagonal stationary: partitions 34g..34g+33 hold row-parity g's
    64 dims (DoubleRow pairs) + 3-term fp8 norm split (ra, rb, rc).
  - PE: DoubleRow fp8 matmuls; 62 compute tiles of 1024 psum cols across
    4 in-flight psum groups (breaks the PE->reader->PE latency chain).
  - per-tile stream DMAs ([68, 2KB]), all on the GP/SWDGE queue with a
    20-slot ring throttle (SWDGE spreads over all 16 DMA engines; the
    SP/ACT HWDGE queues are slow and only reach engines 0-3).
  - pooling into "scattered windows" of 4 psum cols {w, w+256, w+512,
    w+768} per tile: P1 tiles = direct DVE strided tensor_reduce from
    psum; P2 tiles = ACT psum->bf16 copy + 2-level flat-halves DVE
    tt-max tree (both levels hit the 2x DVE mode).
  - pooled [128, 15872] bf16 DMA'd out in 8 chunks on the GP queue.
Host: merge 8 cores' windows, top-48 per obs row, exact float64 re-score
of their rows, top-16, ret-sum argmax, gather action.
"""
from contextlib import ExitStack

import numpy as np
import ml_dtypes

import concourse.bass as bass
from concourse import mybir
from concourse.bass_utils import run_bass_kernel_spmd

F32 = mybir.dt.float32
BF16 = mybir.dt.bfloat16
FP8 = mybir.dt.float8e4
E4 = ml_dtypes.float8_e4m3
BF = ml_dtypes.bfloat16

# problem constants
N_MEMS = 1_000_000
MEM_DIM = 88
B = 64
D = 64
ACT_LEN = 16
RET_LEN = 8
K_TOP = 16
N_CORES = 8

# kernel geometry
KDIM = 68                      # 2 groups x (32 dim-pairs + 2 norm partitions)
TILES = 62                     # compute tiles
TCOLS = 1024                   # psum columns per tile
NCOLS = TILES * TCOLS          # 63488 columns/core (2 rows per column)
R_SHARD = N_MEMS // N_CORES    # 125000
NWIN_T = 256                   # windows per tile (4 scattered cols each)
NPOOL = TILES * NWIN_T         # 15872
RBUF = 20                      # stream ring (per-tile DMA slots)
CB = 4                         # ACT copy ring (tiles)
PAD_R = 240.0                  # pad sentinel for ra/rb (score ~ -480)
HOST_TOPW = 48
P1_TILES = frozenset(round(i * 58 / 15) for i in range(15)) | {59, 60, 61}


def _build_module():
    nc = bass.Bass()
    w_dram = nc.dram_tensor("w", [KDIM, 2 * 128], FP8, kind="ExternalInput")
    x_dram = nc.dram_tensor("x", [KDIM, 2 * NCOLS], FP8, kind="ExternalInput")
    pooled_dram = nc.dram_tensor("pooled", [128, NPOOL], BF16,
                                 kind="ExternalOutput")

    p2_list = [t for t in range(TILES) if t not in P1_TILES]
    p2_index = {t: i for i, t in enumerate(p2_list)}

    with ExitStack() as ctx:
        w_sb = ctx.enter_context(nc.sbuf_tensor("w_sb", [KDIM, 2 * 128], FP8))
        xb = [ctx.enter_context(nc.sbuf_tensor(f"xb{i}", [KDIM, 2048], FP8))
              for i in range(RBUF)]
        cb = [ctx.enter_context(nc.sbuf_tensor(f"cb{i}", [128, TCOLS], BF16))
              for i in range(CB)]
        d1 = [ctx.enter_context(nc.sbuf_tensor(f"d1_{i}", [128, TCOLS // 2],
                                               BF16)) for i in range(2)]
        pooled = ctx.enter_context(nc.sbuf_tensor("pooled_sb", [128, NPOOL],
                                                  BF16))
        ps = [ctx.enter_context(nc.psum_tensor(f"ps{i}", [128, TCOLS], F32))
              for i in range(4)]
        s_w = ctx.enter_context(nc.semaphore("s_w"))
        s_x = [ctx.enter_context(nc.semaphore(f"s_x{i}")) for i in range(RBUF)]
        s_pe = ctx.enter_context(nc.semaphore("s_pe"))
        s_rd = [ctx.enter_context(nc.semaphore(f"s_rd{i}")) for i in range(4)]
        s_pool = ctx.enter_context(nc.semaphore("s_pool"))
        blk = ctx.enter_context(nc.Block())

        def load_dma(eng, t):
            # one DMA per compute tile: [68, 2KB] full-partition (narrow
            # DMAs collapse onto one engine; full ones fan out). The ring
            # throttle keeps in-flight DMAs bounded, which also keeps the
            # engines focused on the stream front.
            if t >= RBUF:
                eng.wait_ge(s_pe, t - RBUF + 1)
            c0 = t * 2048
            eng.dma_start(xb[t % RBUF][:], x_dram[:, c0:c0 + 2048]
                          ).then_inc(s_x[t % RBUF], 16)

        def on_sp(t):
            # entire stream on the (wide) SWDGE queue; the HWDGE queues
            # are slow per descriptor and caused mid-run stalls
            return False

        @blk.sync
        def _(sync):
            pass

        @blk.tensor
        def _(pe):
            pe.wait_ge(s_w, 16)
            w_ap = w_sb[:].rearrange("p (two m) -> p two m", two=2)
            for t in range(TILES):
                pe.wait_ge(s_x[t % RBUF], 16 * (t // RBUF + 1))
                if t >= 4:
                    pe.wait_ge(s_rd[t % 4], t // 4)
                pst = ps[t % 4]
                xbt = xb[t % RBUF]
                last = None
                for c in range(2):
                    xa = xbt[:, c * 1024:(c + 1) * 1024].rearrange(
                        "p (two n) -> p two n", two=2)
                    last = pe.matmul(pst[:, c * 512:(c + 1) * 512], w_ap, xa,
                                     start=True, stop=True,
                                     perf_mode=mybir.MatmulPerfMode.DoubleRow)
                last.then_inc(s_pe, 1)

        @blk.scalar
        def _(act):
            # s_pool counts completed P2 trees in P2 order:
            # `s_pool >= i-CB+1` means cbuf slot i%CB is drained.
            for i, t in enumerate(p2_list):
                act.wait_ge(s_pe, t + 1)
                if i >= CB:
                    act.wait_ge(s_pool, i - (CB - 1))
                act.copy(cb[i % CB][:], ps[t % 4][:]
                         ).then_inc(s_rd[t % 4], 1)

        @blk.vector
        def _(dve):
            for t in range(TILES):
                po = pooled[:, t * NWIN_T:(t + 1) * NWIN_T]
                if t in P1_TILES:
                    dve.wait_ge(s_pe, t + 1)
                    # scattered window {w, w+256, w+512, w+768}
                    dve.tensor_reduce(
                        po, ps[t % 4][:].rearrange("p (w nw) -> p nw w", w=4),
                        axis=mybir.AxisListType.X, op=mybir.AluOpType.max,
                        opt_input=False,
                    ).then_inc(s_rd[t % 4], 1)
                else:
                    i = p2_index[t]
                    dve.wait_ge(s_rd[t % 4], t // 4 + 1)
                    cbt = cb[i % CB]
                    d1t = d1[i % 2]
                    dve.tensor_tensor(d1t[:], cbt[:, 0:512], cbt[:, 512:1024],
                                      op=mybir.AluOpType.max)
                    dve.tensor_tensor(po, d1t[:, 0:256], d1t[:, 256:512],
                                      op=mybir.AluOpType.max
                                      ).then_inc(s_pool, 1)

        @blk.gpsimd
        def _(gp):
            gp.dma_start(w_sb[:], w_dram[:]).then_inc(s_w, 16)
            for t in range(TILES):
                if not on_sp(t):
                    load_dma(gp, t)
            # pooled output in 16 chunks. Tiles [0, hi) are pooled when the
            # four bank sems cover all consumed tiles < hi (P1 reduce ==
            # pooled) and s_pool covers every P2 tree with tile < hi.
            for k in range(8):
                hi = min(8 * (k + 1), TILES)           # tiles [8k, hi)
                n_p2 = sum(1 for t in p2_list if t < hi)
                for j in range(4):
                    gp.wait_ge(s_rd[j], (hi - j + 3) // 4)
                gp.wait_ge(s_pool, n_p2)
                lo_c, hi_c = 8 * k * NWIN_T, hi * NWIN_T
                gp.dma_start(pooled_dram[:, lo_c:hi_c], pooled[:, lo_c:hi_c]
                             ).then_inc(s_w, 16)

    return nc


# ---------------- host side ----------------

def _pack_inputs(obs: np.ndarray, memories: np.ndarray):
    """Returns (w_packed [68,256] fp8, [x_core fp8 [68, 2*NCOLS]] * 8)."""
    norm = np.clip(np.linalg.norm(obs, axis=1, keepdims=True), 1e-12, None)
    obs_n = (obs / norm).astype(np.float32)
    wq = (2.0 * obs_n).astype(E4)                      # [64 obs, 64 dims]

    w = np.zeros((KDIM, 2, 128), dtype=E4)
    for g in range(2):
        # dims: partition 34g+kk, slot j -> dim 2kk+j ; obs m -> out 64g+m
        w[34 * g:34 * g + 32, :, 64 * g:64 * g + 64] = (
            wq.T.reshape(32, 2, 64))
        w[34 * g + 32, :, 64 * g:64 * g + 64] = E4(-1.0)
        w[34 * g + 33, 0, 64 * g:64 * g + 64] = E4(-1.0)
    w_packed = np.ascontiguousarray(w.reshape(KDIM, 256))

    m64 = memories[:, :D].astype(np.float32)
    norms2 = np.einsum("nd,nd->n", m64, m64, dtype=np.float32)
    r = norms2 - np.float32(64.0)
    ra = r.astype(E4)
    rb = (r - ra.astype(np.float32)).astype(E4)
    rc = (r - ra.astype(np.float32) - rb.astype(np.float32)).astype(E4)
    mq = m64.astype(E4)

    xs = []
    for c in range(N_CORES):
        lo = c * R_SHARD
        rows = np.zeros((2 * NCOLS, 64), dtype=E4)
        rows[:R_SHARD] = mq[lo:lo + R_SHARD]
        rav = np.full(2 * NCOLS, PAD_R, dtype=E4)
        rbv = np.full(2 * NCOLS, PAD_R, dtype=E4)
        rcv = np.zeros(2 * NCOLS, dtype=E4)
        rav[:R_SHARD] = ra[lo:lo + R_SHARD]
        rbv[:R_SHARD] = rb[lo:lo + R_SHARD]
        rcv[:R_SHARD] = rc[lo:lo + R_SHARD]

        x = np.empty((KDIM, 124, 2, 512), dtype=E4)    # [k, chunk, j, n]
        # rows -> [chunk, n, g, kk, j]
        r5 = rows.reshape(124, 512, 2, 32, 2)
        rd = r5.transpose(2, 3, 0, 4, 1)               # [g, kk, ch, j, n]
        ra3 = rav.reshape(124, 512, 2)                 # [ch, n, g]
        rb3 = rbv.reshape(124, 512, 2)
        rc3 = rcv.reshape(124, 512, 2)
        for g in range(2):
            x[34 * g:34 * g + 32] = rd[g]
            x[34 * g + 32, :, 0, :] = ra3[:, :, g]
            x[34 * g + 32, :, 1, :] = rb3[:, :, g]
            x[34 * g + 33, :, 0, :] = rc3[:, :, g]
            x[34 * g + 33, :, 1, :] = E4(0.0)
        xs.append(np.ascontiguousarray(x.reshape(KDIM, 2 * NCOLS)))
    return w_packed, xs


def _finalize(memories: np.ndarray, obs: np.ndarray,
              pooled: np.ndarray) -> np.ndarray:
    """pooled: [n_cores, 128, NPOOL] bf16-as-float -> best_acts [B, ACT_LEN]."""
    obs_n = obs.astype(np.float64)
    obs_n /= np.clip(np.linalg.norm(obs_n, axis=1, keepdims=True), 1e-12, None)

    # candidate windows: [B, cores, 2, NPOOL]; window (t, w) covers psum
    # cols {t*1024 + w + 256j, j<4} -> rows 2*col + g
    cand = np.empty((B, N_CORES, 2, NPOOL), dtype=np.float32)
    for c in range(N_CORES):
        pc = pooled[c].astype(np.float32)
        cand[:, c, 0] = pc[0:64]
        cand[:, c, 1] = pc[64:128]
    flat = cand.reshape(B, -1)
    top = np.argpartition(-flat, HOST_TOPW, axis=1)[:, :HOST_TOPW]

    mem64 = memories[:, :D]
    offs = 256 * np.arange(4, dtype=np.int64)
    best_acts = np.empty((B, ACT_LEN), dtype=np.float32)
    for b in range(B):
        wins = top[b]
        core = wins // (2 * NPOOL)
        g = (wins // NPOOL) % 2
        wi = wins % NPOOL
        t = wi // NWIN_T
        w = wi % NWIN_T
        cols = (t * TCOLS + w)[:, None] + offs[None, :]   # [W, 4]
        local = 2 * cols.ravel() + np.repeat(g, 4)
        cs = np.repeat(core, 4)
        valid = local < R_SHARD
        rows = np.unique(cs[valid] * R_SHARD + local[valid])
        cm = mem64[rows].astype(np.float64)
        d2 = ((cm * cm).sum(axis=1) - 2.0 * (cm @ obs_n[b])
              + (obs_n[b] * obs_n[b]).sum())
        order = np.argsort(d2, kind="stable")[:K_TOP]
        top_rows = rows[order]
        ret_sum = memories[top_rows, D + ACT_LEN:].astype(np.float64).sum(axis=1)
        best = int(np.argmax(ret_sum))
        best_acts[b] = memories[top_rows[best], D:D + ACT_LEN]
    return best_acts


_CACHED_NC = None


def run_knn(inputs: dict, trace: bool = False):
    global _CACHED_NC
    obs = np.asarray(inputs["obs"], dtype=np.float32)
    memories = np.asarray(inputs["memories"], dtype=np.float32)
    assert obs.shape == (B, D) and memories.shape == (N_MEMS, MEM_DIM)
    assert int(inputs["obs_len"]) == D and int(inputs["act_len"]) == ACT_LEN
    assert int(inputs["k"]) == K_TOP

    w_packed, xs = _pack_inputs(obs, memories)
    in_maps = [{"w": w_packed, "x": xs[c]} for c in range(N_CORES)]

    if _CACHED_NC is None:
        _CACHED_NC = _build_module()
    res = run_bass_kernel_spmd(_CACHED_NC, in_maps,
                               core_ids=list(range(N_CORES)), trace=trace)
    pooled = np.stack([np.asarray(r["pooled"]) for r in res.results])
    out = _finalize(memories, obs, pooled)
    return out, res.exec_time_ns


def kernel(**inputs) -> np.ndarray:
    out, _ = run_knn(inputs, trace=False)
    return out
